# revision 12
# baseline (speedup 1.0000x reference)
"""BiAttentionMRU Trainium2 kernel.

Data-parallel over batch: B=16 -> 2 batch elements on each of 8 cores.
All weights replicated. Embedding gather done on-device via indirect DMA.

Layouts (per core, per batch element b in {0,1}):
  art gathered as [t-chunk(128), d=300], PE-transposed into artT[dc][100, 2000]
  (d on partitions, 3 chunks of 100). Group sums, z/o/CE matmuls, gate mix,
  MRU scan (native tensor_tensor_scan along t) and the attention lhsT all
  work in [d, t] layout.

Attention algebra: aoq is never materialized. With e1 = exp(art_enc @ keys1^T),
Z1 its row sum, s2 = softmax-normalized p1 @ (q @ keys_f^T) is computed as
exp-of(u2 * 1/Z1) where u2 = e1 @ QK. The per-option mean over t of
softmax(s2) @ opt folds into one accumulating matmul sum_t e2[t,:] * (1/Z2[t]).
"""

import sys

sys.path.insert(0, "/opt/trn_rl_repo")

import numpy as np
import ml_dtypes

import concourse.bass as bass
import concourse.tile as tile
from concourse import bacc, mybir
from concourse.masks import make_identity

F32 = mybir.dt.float32
BF16 = mybir.dt.bfloat16
I32 = mybir.dt.int32
AX = mybir.AxisListType
OP = mybir.AluOpType
AF = mybir.ActivationFunctionType

DIM = 300
VOCAB = 50000
B_FULL = 16
NCORES = 8
BPC = B_FULL // NCORES  # batch per core = 2
T = 2000
TQ = 30
TO = 16
RANGES = (1, 2, 4, 10, 25)

TCH = [128] * 15 + [80]  # t chunking for transposes / attention
NTCH = len(TCH)
DC = 3  # d chunks of 100
DCS = 100

N_MM = 500  # matmul N-chunking for [d,t] streams (psum free <= 512 fp32)

USE_BF16 = True
DT = BF16 if USE_BF16 else F32
NPDT = ml_dtypes.bfloat16 if USE_BF16 else np.float32

# scalar table columns (host-packed, replicated down 128 partitions)
SC_M1 = 0      # 15 cols: m1[k,r]/r at 5k+ri
SC_M1B = 15    # 3 cols
SC_M2 = 18     # 3 cols
SC_M2B = 21    # 1 col
SC_AS2B = 22   # 1 col
SC_NCOL = 24


def _build_program():
    nc = bacc.Bacc("TRN2", target_bir_lowering=False, debug=False,
                   num_devices=NCORES)

    emb = nc.dram_tensor("emb", [VOCAB, DIM], DT, kind="ExternalInput")
    emb32 = nc.dram_tensor("emb32", [VOCAB, DIM], F32, kind="ExternalInput")
    art_idx = nc.dram_tensor("art_idx", [BPC, T], I32, kind="ExternalInput")
    q_idx = nc.dram_tensor("q_idx", [BPC, TQ], I32, kind="ExternalInput")
    opt_idx = nc.dram_tensor("opt_idx", [BPC, 4, TO], I32, kind="ExternalInput")
    w_art = nc.dram_tensor("w_art", [DIM, 3 * DIM], DT, kind="ExternalInput")
    w_ce = nc.dram_tensor("w_ce", [4, DIM, DIM], DT, kind="ExternalInput")
    w_f1 = nc.dram_tensor("w_f1", [DIM, DIM], DT, kind="ExternalInput")
    w_f2 = nc.dram_tensor("w_f2", [DIM, DIM], DT, kind="ExternalInput")
    w_f3 = nc.dram_tensor("w_f3", [DIM, DIM], DT, kind="ExternalInput")
    w_as1 = nc.dram_tensor("w_as1", [2 * DIM, 75], F32, kind="ExternalInput")
    w_as2 = nc.dram_tensor("w_as2", [75, 1], F32, kind="ExternalInput")
    # bias cols: 0 bz, 1 bo, 2..6 ce_b[0..4], 7 f1_b, 8 f2_b, 9 f3_b
    biases = nc.dram_tensor("biases", [DIM, 10], F32, kind="ExternalInput")
    b_as1 = nc.dram_tensor("b_as1", [75, 1], F32, kind="ExternalInput")
    scal = nc.dram_tensor("scal", [128, SC_NCOL], F32, kind="ExternalInput")
    scal_dt = nc.dram_tensor("scal_dt", [128, SC_NCOL], DT, kind="ExternalInput")
    out = nc.dram_tensor("scores", [BPC, 4], F32, kind="ExternalOutput")

    with tile.TileContext(nc) as tc:
        from contextlib import ExitStack
        with ExitStack() as ctx:
            _emit(nc, tc, ctx, emb, emb32, art_idx, q_idx, opt_idx, w_art, w_ce,
                  w_f1, w_f2, w_f3, w_as1, w_as2, biases, b_as1, scal,
                  scal_dt, out)

    nc.compile()
    return nc


def _emit(nc, tc, ctx, emb, emb32, art_idx, q_idx, opt_idx, w_art, w_ce, w_f1,
          w_f2, w_f3, w_as1, w_as2, biases, b_as1, scal, scal_dt, out):
    # ---------------- pools ----------------
    consts = ctx.enter_context(tc.tile_pool(name="consts", bufs=1))
    gpool = ctx.enter_context(tc.tile_pool(name="gather", bufs=6))
    p_art = ctx.enter_context(tc.tile_pool(name="p_art", bufs=2))
    p_enc = ctx.enter_context(tc.tile_pool(name="p_enc", bufs=2))
    p_zb = ctx.enter_context(tc.tile_pool(name="p_zb", bufs=1))
    p_mix = ctx.enter_context(tc.tile_pool(name="p_mix", bufs=2))
    p_xs = ctx.enter_context(tc.tile_pool(name="p_xs", bufs=1))
    small = ctx.enter_context(tc.tile_pool(name="small", bufs=3))
    pp500 = ctx.enter_context(tc.tile_pool(name="pp500", bufs=2, space="PSUM"))
    pptr = ctx.enter_context(tc.tile_pool(name="pptr", bufs=2, space="PSUM"))
    ppatt = ctx.enter_context(tc.tile_pool(name="ppatt", bufs=2, space="PSUM"))
    ppacc = ctx.enter_context(tc.tile_pool(name="ppacc", bufs=2, space="PSUM"))

    # ---------------- constants / weights ----------------
    ident = consts.tile([128, 128], DT)
    make_identity(nc, ident[:])

    w_art_sb = consts.tile([DCS, DC, 3 * DIM], DT)
    for kc in range(DC):
        nc.sync.dma_start(w_art_sb[:, kc, :], w_art[kc * DCS:(kc + 1) * DCS, :])
    w_ce_sb = consts.tile([DCS, DC, 4, DIM], DT)
    for kc in range(DC):
        for ri in range(4):
            nc.sync.dma_start(w_ce_sb[:, kc, ri, :],
                              w_ce[ri, kc * DCS:(kc + 1) * DCS, :])
    w_f1_sb = consts.tile([DCS, DC, DIM], DT)
    w_f2_sb = consts.tile([DCS, DC, DIM], DT)
    w_f3_sb = consts.tile([DCS, DC, DIM], DT)
    for w_sb, w_dram in ((w_f1_sb, w_f1), (w_f2_sb, w_f2), (w_f3_sb, w_f3)):
        for kc in range(DC):
            nc.sync.dma_start(w_sb[:, kc, :], w_dram[kc * DCS:(kc + 1) * DCS, :])
    w_as1_sb = consts.tile([DCS, 6, 75], F32)
    for j in range(6):
        nc.sync.dma_start(w_as1_sb[:, j, :], w_as1[j * DCS:(j + 1) * DCS, :])
    w_as2_sb = consts.tile([75, 1], F32)
    nc.sync.dma_start(w_as2_sb[:], w_as2[:])
    bias_sb = consts.tile([DCS, DC, 10], F32)
    for kc in range(DC):
        nc.sync.dma_start(bias_sb[:, kc, :], biases[kc * DCS:(kc + 1) * DCS, :])
    b_as1_sb = consts.tile([75, 1], F32)
    nc.sync.dma_start(b_as1_sb[:], b_as1[:])
    scal_sb = consts.tile([128, SC_NCOL], F32)
    nc.sync.dma_start(scal_sb[:], scal[:])
    scal_dt_sb = consts.tile([128, SC_NCOL], DT)
    nc.sync.dma_start(scal_dt_sb[:], scal_dt[:])

    def sc(col):  # f32 per-partition scalar [100,1]
        return scal_sb[0:DCS, col:col + 1]

    ans_sb = small.tile([DCS, BPC, 6, 4], F32, tag="ans_sb")

    for b in range(BPC):
        # ---------------- indices ----------------
        aidx = small.tile([128, NTCH], I32, tag="aidx")
        for c in range(NTCH):
            pc = TCH[c]
            nc.sync.dma_start(aidx[:pc, c:c + 1],
                              art_idx[b, c * 128:c * 128 + pc, None])
        qidx = small.tile([TQ, 1], I32, tag="qidx")
        nc.sync.dma_start(qidx[:], q_idx[b, :, None])
        oidx = small.tile([TO, 4], I32, tag="oidx")
        nc.sync.dma_start(oidx[:], opt_idx[b].rearrange("o w -> w o"))

        # ---------------- gathers + transposes ----------------
        artT = [p_art.tile([DCS, T], DT, tag=f"artT{dc}", name=f"artT{dc}") for dc in range(DC)]
        for c in range(NTCH):
            pc = TCH[c]
            g = gpool.tile([128, DIM], DT, tag="gart")
            nc.gpsimd.indirect_dma_start(
                out=g[:pc, :], out_offset=None, in_=emb[:],
                in_offset=bass.IndirectOffsetOnAxis(ap=aidx[:pc, c:c + 1], axis=0))
            for dc in range(DC):
                tp = pptr.tile([DCS, 128], DT, tag="tr")
                nc.tensor.transpose(tp[:, :pc], g[:pc, dc * DCS:(dc + 1) * DCS],
                                    ident[:pc, :pc])
                nc.vector.tensor_copy(artT[dc][:, c * 128:c * 128 + pc],
                                      tp[:, :pc])

        qg = small.tile([TQ, DIM], DT, tag="qg")
        nc.gpsimd.indirect_dma_start(
            out=qg[:], out_offset=None, in_=emb[:],
            in_offset=bass.IndirectOffsetOnAxis(ap=qidx[:, 0:1], axis=0))
        qT = small.tile([DCS, DC, TQ], DT, tag="qT")
        for dc in range(DC):
            tp = pptr.tile([DCS, 128], DT, tag="tr")
            nc.tensor.transpose(tp[:, :TQ], qg[:, dc * DCS:(dc + 1) * DCS],
                                ident[:TQ, :TQ])
            nc.vector.tensor_copy(qT[:, dc, :], tp[:, :TQ])

        og = [small.tile([TO, DIM], F32, tag=f"og{o}", name=f"og{o}") for o in range(4)]
        ogb = small.tile([TO, 4, DIM], DT, tag="ogb")
        oT = small.tile([DCS, DC, 4, TO], DT, tag="oT")
        for o in range(4):
            nc.gpsimd.indirect_dma_start(
                out=og[o][:], out_offset=None, in_=emb32[:],
                in_offset=bass.IndirectOffsetOnAxis(ap=oidx[:, o:o + 1], axis=0))
            nc.vector.tensor_copy(ogb[:, o, :], og[o][:])
            for dc in range(DC):
                tp = pptr.tile([DCS, 128], DT, tag="tr")
                nc.tensor.transpose(tp[:, :TO], ogb[:, o, dc * DCS:(dc + 1) * DCS],
                                    ident[:TO, :TO])
                nc.vector.tensor_copy(oT[:, dc, o, :], tp[:, :TO])

        # ---------------- group sums (xs_r in [d, g] layout) ----------------
        xs2 = [p_xs.tile([DCS, T // 2], DT, tag=f"xs2_{dc}", name=f"xs2_{dc}") for dc in range(DC)]
        xs4 = [p_xs.tile([DCS, T // 4], DT, tag=f"xs4_{dc}", name=f"xs4_{dc}") for dc in range(DC)]
        xs10 = [p_xs.tile([DCS, T // 10], DT, tag=f"xs10_{dc}", name=f"xs10_{dc}") for dc in range(DC)]
        xs25 = [p_xs.tile([DCS, T // 25], DT, tag=f"xs25_{dc}", name=f"xs25_{dc}") for dc in range(DC)]
        for dc in range(DC):
            a = artT[dc]
            nc.gpsimd.tensor_add(xs2[dc][:], a[:, 0:T:2], a[:, 1:T:2])
            nc.gpsimd.tensor_add(xs4[dc][:], xs2[dc][:, 0:T // 2:2],
                                 xs2[dc][:, 1:T // 2:2])
            with nc.allow_low_precision(reason="bf16 group sums feed bf16 matmuls"):
                nc.vector.tensor_reduce(
                    xs10[dc][:], xs2[dc][:].rearrange("p (g r) -> p g r", r=5),
                    AX.X, OP.add)
                nc.vector.tensor_reduce(
                    xs25[dc][:], a[:].rearrange("p (g r) -> p g r", r=25),
                    AX.X, OP.add)

        # ---------------- z / o / B1 (art stream) ----------------
        z_sb = [p_zb.tile([DCS, T], DT, tag=f"z{dc}", name=f"z{dc}") for dc in range(DC)]
        o_sb = [p_enc.tile([DCS, T], DT, tag=f"o{dc}", name=f"o{dc}") for dc in range(DC)]
        b1_sb = [p_zb.tile([DCS, T], DT, tag=f"b1_{dc}", name=f"b1_{dc}") for dc in range(DC)]
        for mi, (dst, func, bcol) in enumerate(
                ((z_sb, AF.Tanh, 0), (o_sb, AF.Tanh, 1), (b1_sb, AF.Relu, 2))):
            for dc in range(DC):
                mcol = mi * DIM + dc * DCS
                for t0 in range(0, T, N_MM):
                    ps = pp500.tile([DCS, N_MM], F32, tag="mm")
                    for kc in range(DC):
                        nc.tensor.matmul(
                            ps[:], w_art_sb[:, kc, mcol:mcol + DCS],
                            artT[kc][:, t0:t0 + N_MM],
                            start=(kc == 0), stop=(kc == DC - 1))
                    nc.scalar.activation(dst[dc][:, t0:t0 + N_MM], ps[:],
                                         func, bias=bias_sb[:, dc, bcol:bcol + 1])

        # ---------------- CE r>=2 ----------------
        bl = {}
        for ri, (xs, r) in enumerate(((xs2, 2), (xs4, 4), (xs10, 10), (xs25, 25))):
            g_r = T // r
            bl[r] = [p_xs.tile([DCS, g_r], DT, tag=f"bl{r}_{dc}", name=f"bl{r}_{dc}")
                     for dc in range(DC)]
            for dc in range(DC):
                for g0 in range(0, g_r, N_MM):
                    gn = min(N_MM, g_r - g0)
                    ps = pp500.tile([DCS, N_MM], F32, tag="mm")
                    for kc in range(DC):
                        nc.tensor.matmul(
                            ps[:, :gn],
                            w_ce_sb[:, kc, ri, dc * DCS:(dc + 1) * DCS],
                            xs[kc][:, g0:g0 + gn],
                            start=(kc == 0), stop=(kc == DC - 1))
                    nc.scalar.activation(bl[r][dc][:, g0:g0 + gn], ps[:, :gn],
                                         AF.Relu, bias=bias_sb[:, dc, 3 + ri:4 + ri])

        # ---------------- gate mix ----------------
        gate = []
        for dc in range(DC):
            ev = [b1_sb[dc][:]]
            for r in (2, 4, 10, 25):
                ev.append(bl[r][dc][:, :, None].to_broadcast([DCS, T // r, r]))
            h1 = []
            for k in range(3):
                eng = nc.vector  # scalar_tensor_tensor is DVE-only
                acc = p_mix.tile([DCS, T], DT, tag=f"h1_{k}")
                eng.scalar_tensor_tensor(
                    acc[:], ev[0], sc(SC_M1 + 5 * k),
                    scal_dt_sb[0:DCS, SC_M1B + k:SC_M1B + k + 1]
                    .to_broadcast([DCS, T]),
                    op0=OP.mult, op1=OP.add)
                for ri in range(1, 5):
                    eng.scalar_tensor_tensor(
                        acc[:], ev[ri], sc(SC_M1 + 5 * k + ri), acc[:],
                        op0=OP.mult, op1=OP.add)
                nc.scalar.activation(acc[:], acc[:], AF.Relu)
                h1.append(acc)
            g_acc = p_mix.tile([DCS, T], DT, tag="gate")
            nc.vector.scalar_tensor_tensor(
                g_acc[:], h1[0][:], sc(SC_M2),
                scal_dt_sb[0:DCS, SC_M2B:SC_M2B + 1].to_broadcast([DCS, T]),
                op0=OP.mult, op1=OP.add)
            nc.vector.scalar_tensor_tensor(
                g_acc[:], h1[1][:], sc(SC_M2 + 1), g_acc[:],
                op0=OP.mult, op1=OP.add)
            nc.vector.scalar_tensor_tensor(
                g_acc[:], h1[2][:], sc(SC_M2 + 2), g_acc[:],
                op0=OP.mult, op1=OP.add)
            nc.scalar.activation(g_acc[:], g_acc[:], AF.Relu)
            gate.append(g_acc)

        # ---------------- MRU scan + encode ----------------
        encT = []
        for dc in range(DC):
            gz = p_mix.tile([DCS, T], DT, tag="gz")
            nc.gpsimd.tensor_tensor(gz[:], gate[dc][:], z_sb[dc][:], op=OP.mult)
            nc.vector.tensor_sub(z_sb[dc][:], z_sb[dc][:], gz[:])  # (1-g)z
            c_t = p_mix.tile([DCS, T], DT, tag="c")
            nc.vector.tensor_tensor_scan(
                c_t[:], gate[dc][:], z_sb[dc][:], 0.0, op0=OP.mult, op1=OP.add)
            nc.vector.tensor_mul(o_sb[dc][:], o_sb[dc][:], c_t[:])
            encT.append(o_sb[dc])

        # ---------------- keys1T ----------------
        k1T = small.tile([DCS, DC, TQ], DT, tag="k1T")
        for dc in range(DC):
            ps = ppatt.tile([DCS, TQ], F32, tag="att")
            for kc in range(DC):
                nc.tensor.matmul(ps[:], w_f1_sb[:, kc, dc * DCS:(dc + 1) * DCS],
                                 qT[:, kc, :], start=(kc == 0), stop=(kc == DC - 1))
            nc.scalar.copy(k1T[:, dc, :], ps[:])

        # ---------------- A2/A3 and QK ----------------
        aTs = []
        for fi, w_f_sb in enumerate((w_f2_sb, w_f3_sb)):
            a_ps = ppatt.tile([TQ, DIM], F32, tag="att")
            for kc in range(DC):
                nc.tensor.matmul(a_ps[:], qT[:, kc, :], w_f_sb[:, kc, :],
                                 start=(kc == 0), stop=(kc == DC - 1))
            a_sb = small.tile([TQ, DIM], DT, tag="a_sb")
            nc.vector.tensor_copy(a_sb[:], a_ps[:])
            aT = small.tile([DCS, DC, TQ], DT, tag=f"aT{fi}")
            for dc in range(DC):
                tp = pptr.tile([DCS, 128], DT, tag="tr")
                nc.tensor.transpose(tp[:, :TQ], a_sb[:, dc * DCS:(dc + 1) * DCS],
                                    ident[:TQ, :TQ])
                nc.vector.tensor_copy(aT[:, dc, :], tp[:, :TQ])
            aTs.append(aT)

        qk_ps = ppacc.tile([TQ, 128], F32, tag="acc")
        for fi in range(2):
            for o in range(4):
                gcol = 16 * (4 * fi + o)
                for kc in range(DC):
                    nc.tensor.matmul(qk_ps[:, gcol:gcol + 16],
                                     aTs[fi][:, kc, :], oT[:, kc, o, :],
                                     start=(kc == 0), stop=(kc == DC - 1))
        qk_sb = small.tile([TQ, 128], DT, tag="qk_sb")
        nc.vector.tensor_copy(qk_sb[:], qk_ps[:])

        # ---------------- attention stream over t chunks ----------------
        pb_ps = ppacc.tile([128, 8], F32, tag="acc")
        for c in range(NTCH):
            pc = TCH[c]
            s1 = ppatt.tile([128, TQ], F32, tag="att")
            for dc in range(DC):
                nc.tensor.matmul(s1[:pc, :], encT[dc][:, c * 128:c * 128 + pc],
                                 k1T[:, dc, :], start=(dc == 0),
                                 stop=(dc == DC - 1))
            e1 = small.tile([128, TQ], DT, tag="e1")
            z1 = small.tile([128, 2], F32, tag="z1")
            nc.scalar.activation(e1[:pc, :], s1[:pc, :], AF.Exp,
                                 accum_out=z1[:pc, 0:1])
            nc.vector.reciprocal(z1[:pc, 1:2], z1[:pc, 0:1])
            tp = pptr.tile([TQ, 128], DT, tag="tr")
            nc.tensor.transpose(tp[:, :pc], e1[:pc, :], ident[:pc, :pc])
            e1T = small.tile([TQ, 128], DT, tag="e1Ts")
            nc.vector.tensor_copy(e1T[:, :pc], tp[:, :pc])
            u2 = ppatt.tile([128, 128], F32, tag="att")
            nc.tensor.matmul(u2[:pc, :], e1T[:, :pc], qk_sb[:],
                             start=True, stop=True)
            e2 = small.tile([128, 128], DT, tag="e2")
            nc.scalar.activation(e2[:pc, :], u2[:pc, :], AF.Exp,
                                 scale=z1[:pc, 1:2])
            z2 = small.tile([128, 16], F32, tag="z2")
            nc.vector.tensor_reduce(z2[:pc, 0:8],
                                    e2[:pc, :].rearrange("p (g w) -> p g w", w=16),
                                    AX.X, OP.add)
            nc.vector.reciprocal(z2[:pc, 8:16], z2[:pc, 0:8])
            rz2 = small.tile([128, 8], DT, tag="rz2")
            # fold the mean-over-t (1/T) into the reduction weights
            nc.vector.tensor_scalar_mul(rz2[:pc, :], z2[:pc, 8:16], 1.0 / T)
            nc.tensor.matmul(pb_ps[:, :], e2[:pc, :], rz2[:pc, :],
                             start=(c == 0), stop=(c == NTCH - 1))

        # ---------------- answer vectors ----------------
        pb_sb = small.tile([128, 8], F32, tag="pb_sb")
        nc.vector.tensor_copy(pb_sb[:], pb_ps[:])
        ans_ps = ppacc.tile([DCS, 24], F32, tag="acc")
        for g in range(8):
            fi, o = g // 4, g % 4
            pb16 = small.tile([TO, 1], F32, tag="pb16")
            nc.sync.dma_start(pb16[:], pb_sb[16 * g:16 * g + 16, g:g + 1])
            for dc in range(DC):
                j = fi * 3 + dc
                nc.tensor.matmul(ans_ps[:, j * 4 + o:j * 4 + o + 1],
                                 og[o][:, dc * DCS:(dc + 1) * DCS], pb16[:],
                                 start=True, stop=True)
        nc.vector.tensor_copy(ans_sb[:, b, :, :].rearrange("p j o -> p (j o)"),
                              ans_ps[:])

    # ---------------- final MLP (both batches together) ----------------
    h_ps = ppatt.tile([75, 8], F32, tag="att")
    for j in range(6):
        # rhs columns = (b, o) pairs for chunk j of the 600-dim ans vector
        rhs = ans_sb[:, :, j, :]
        nc.tensor.matmul(h_ps[:], w_as1_sb[:, j, :], rhs,
                         start=(j == 0), stop=(j == 5))
    h_sb = small.tile([75, 8], F32, tag="h_sb")
    nc.scalar.activation(h_sb[:], h_ps[:], AF.Relu, bias=b_as1_sb[:])
    s_ps = ppacc.tile([8, 1], F32, tag="acc")
    nc.tensor.matmul(s_ps[:], h_sb[:], w_as2_sb[:], start=True, stop=True)
    s_sb = small.tile([8, 1], F32, tag="s_sb")
    nc.scalar.activation(s_sb[:], s_ps[:], AF.Identity,
                         bias=scal_sb[0:8, SC_AS2B:SC_AS2B + 1])
    nc.sync.dma_start(out[:].rearrange("b o -> (b o)")[:, None], s_sb[:])


# ---------------------------------------------------------------------------
# host side
# ---------------------------------------------------------------------------

_CACHE = {}


def _get_nc():
    if "nc" not in _CACHE:
        _CACHE["nc"] = _build_program()
    return _CACHE["nc"]


def _prep_core_inputs(inputs, core):
    b0 = core * BPC
    sl = slice(b0, b0 + BPC)
    f = np.asarray
    prep = _CACHE.get("prep_shared")
    if prep is None:
        # core-independent tensors, computed once per kernel() call set
        Wz, Wo = f(inputs["Wz"]), f(inputs["Wo"])
        ceW = f(inputs["ce_W"])
        prep = {
            "emb": f(inputs["emb"]).astype(NPDT),
            "emb32": f(inputs["emb"]).astype(np.float32),
            "w_art": np.ascontiguousarray(
                np.concatenate([Wz.T, Wo.T, ceW[0].T], axis=1)).astype(NPDT),
            "w_ce": np.ascontiguousarray(
                ceW[1:].transpose(0, 2, 1)).astype(NPDT),
            "w_f1": np.ascontiguousarray(f(inputs["f1_W"]).T).astype(NPDT),
            # s2 = aoq @ f2W @ opt^T, so f2/f3 go in UNtransposed
            # (f1 builds keys1^T = f1W @ q^T and does need the transpose)
            "w_f2": np.ascontiguousarray(f(inputs["f2_W"])).astype(NPDT),
            "w_f3": np.ascontiguousarray(f(inputs["f3_W"])).astype(NPDT),
            "w_as1": np.ascontiguousarray(f(inputs["as1_W"]).T).astype(np.float32),
            "w_as2": np.ascontiguousarray(f(inputs["as2_W"]).T).astype(np.float32),
            "biases": np.stack(
                [f(inputs["bz"]), f(inputs["bo"]),
                 *[f(inputs["ce_b"])[i] for i in range(5)],
                 f(inputs["f1_b"]), f(inputs["f2_b"]), f(inputs["f3_b"])],
                axis=1).astype(np.float32),
            "b_as1": f(inputs["as1_b"])[:, None].astype(np.float32),
        }
        scal = np.zeros((128, SC_NCOL), np.float32)
        m1 = f(inputs["mr1_W"])
        for k in range(3):
            for ri, r in enumerate(RANGES):
                scal[:, SC_M1 + 5 * k + ri] = m1[k, ri] / r
        scal[:, SC_M1B:SC_M1B + 3] = f(inputs["mr1_b"])[None, :]
        scal[:, SC_M2:SC_M2 + 3] = f(inputs["mr2_W"])[0][None, :]
        scal[:, SC_M2B] = f(inputs["mr2_b"])[0]
        scal[:, SC_AS2B] = f(inputs["as2_b"])[0]
        prep["scal"] = scal
        prep["scal_dt"] = scal.astype(NPDT)
        _CACHE["prep_shared"] = prep

    d = dict(prep)
    d["art_idx"] = f(inputs["article_in"])[sl].astype(np.int32)
    d["q_idx"] = f(inputs["question_in"])[sl].astype(np.int32)
    d["opt_idx"] = np.stack(
        [f(inputs[f"option{i}_in"])[sl] for i in (1, 2, 3, 4)],
        axis=1).astype(np.int32)
    return d


def _get_runner():
    """jit-compiled 8-core runner, built once per process."""
    if "runner" in _CACHE:
        return _CACHE["runner"]
    import jax
    from jax.sharding import Mesh, PartitionSpec
    from jax.experimental.shard_map import shard_map
    from concourse.bass2jax import (_bass_exec_p, install_neuronx_cc_hook,
                                    partition_id_tensor)

    install_neuronx_cc_hook()
    nc = _get_nc()
    pid_name = nc.partition_id_tensor.name if nc.partition_id_tensor else None

    in_names, out_names, out_avals, zero_outs = [], [], [], []
    for alloc in nc.m.functions[0].allocations:
        if not isinstance(alloc, mybir.MemoryLocationSet):
            continue
        name = alloc.memorylocations[0].name
        if alloc.kind == "ExternalInput":
            if name != pid_name:
                in_names.append(name)
        elif alloc.kind == "ExternalOutput":
            out_names.append(name)
            shape = tuple(alloc.tensor_shape)
            dtype = mybir.dt.np(alloc.dtype)
            out_avals.append(jax.core.ShapedArray(shape, dtype))
            zero_outs.append(np.zeros(shape, dtype))
    n_params = len(in_names)
    all_in_names = in_names + out_names
    if pid_name is not None:
        all_in_names = all_in_names + [pid_name]

    def _body(*args):
        operands = list(args)
        if pid_name is not None:
            operands.append(partition_id_tensor())
        outs = _bass_exec_p.bind(
            *operands, out_avals=tuple(out_avals), in_names=tuple(all_in_names),
            out_names=tuple(out_names), lowering_input_output_aliases=(),
            sim_require_finite=True, sim_require_nnan=True, nc=nc)
        return tuple(outs)

    devices = jax.devices()[:NCORES]
    mesh = Mesh(np.asarray(devices), ("core",))
    in_specs = (PartitionSpec("core"),) * (n_params + len(out_names))
    out_specs = (PartitionSpec("core"),) * len(out_names)
    sharded = jax.jit(shard_map(_body, mesh=mesh, in_specs=in_specs,
                                out_specs=out_specs, check_rep=False),
                      keep_unused=True)

    _CACHE["runner"] = (sharded, in_names, out_names, zero_outs)
    return _CACHE["runner"]


def run_cores(per_core_inputs):
    """per_core_inputs: list of 8 dicts name->np array. Returns out dicts."""
    sharded, in_names, out_names, zero_outs = _get_runner()
    concat_in = [np.concatenate([per_core_inputs[c][n] for c in range(NCORES)],
                                axis=0) for n in in_names]
    concat_zeros = [np.concatenate([z] * NCORES, axis=0) for z in zero_outs]
    outs = sharded(*concat_in, *concat_zeros)
    result = []
    for c in range(NCORES):
        d = {}
        for i, n in enumerate(out_names):
            arr = np.asarray(outs[i])
            per = arr.shape[0] // NCORES
            d[n] = arr[c * per:(c + 1) * per]
        result.append(d)
    return result


def kernel(**inputs):
    _CACHE.pop("prep_shared", None)
    per_core = [_prep_core_inputs(inputs, c) for c in range(NCORES)]
    res = run_cores(per_core)
    out = np.concatenate([res[c]["scores"] for c in range(NCORES)], axis=0)
    return out.astype(np.float32)


# revision 14
# speedup vs baseline: 134.7122x; 134.7122x over previous
"""BiAttentionMRU Trainium2 kernel.

Data-parallel over batch: B=16 -> 2 batch elements on each of 8 cores.
All weights replicated. Embedding gather done on-device via indirect DMA.

Layouts (per core, per batch element b in {0,1}):
  art gathered as [t-chunk(128), d=300], PE-transposed into artT[dc][100, 2000]
  (d on partitions, 3 chunks of 100). Group sums, z/o/CE matmuls, gate mix,
  MRU scan (native tensor_tensor_scan along t) and the attention lhsT all
  work in [d, t] layout.

Attention algebra: aoq is never materialized. With e1 = exp(art_enc @ keys1^T),
Z1 its row sum, s2 = softmax-normalized p1 @ (q @ keys_f^T) is computed as
exp-of(u2 * 1/Z1) where u2 = e1 @ QK. The per-option mean over t of
softmax(s2) @ opt folds into one accumulating matmul sum_t e2[t,:] * (1/Z2[t]).
"""

import sys

sys.path.insert(0, "/opt/trn_rl_repo")

import numpy as np
import ml_dtypes

import concourse.bass as bass
import concourse.tile as tile
from concourse import bacc, mybir
from concourse.masks import make_identity

F32 = mybir.dt.float32
BF16 = mybir.dt.bfloat16
I32 = mybir.dt.int32
AX = mybir.AxisListType
OP = mybir.AluOpType
AF = mybir.ActivationFunctionType

DIM = 300
VOCAB = 50000
B_FULL = 16
NCORES = 8
BPC = B_FULL // NCORES  # batch per core = 2
T = 2000
TQ = 30
TO = 16
RANGES = (1, 2, 4, 10, 25)

TCH = [128] * 15 + [80]  # t chunking for transposes / attention
NTCH = len(TCH)
DC = 3  # d chunks of 100
DCS = 100

N_MM = 500  # matmul N-chunking for [d,t] streams (psum free <= 512 fp32)

USE_BF16 = True
DT = BF16 if USE_BF16 else F32
NPDT = ml_dtypes.bfloat16 if USE_BF16 else np.float32

# scalar table columns (host-packed, replicated down 128 partitions)
SC_M1 = 0      # 15 cols: m1[k,r]/r at 5k+ri
SC_M1B = 15    # 3 cols
SC_M2 = 18     # 3 cols
SC_M2B = 21    # 1 col
SC_AS2B = 22   # 1 col
SC_NCOL = 24


def _build_program():
    nc = bacc.Bacc("TRN2", target_bir_lowering=False, debug=False,
                   num_devices=NCORES)

    emb = nc.dram_tensor("emb", [VOCAB, DIM], DT, kind="ExternalInput")
    art_idx = nc.dram_tensor("art_idx", [BPC, T], I32, kind="ExternalInput")
    q_idx = nc.dram_tensor("q_idx", [BPC, TQ], I32, kind="ExternalInput")
    opt_idx = nc.dram_tensor("opt_idx", [BPC, 4, TO], I32, kind="ExternalInput")
    w_art = nc.dram_tensor("w_art", [DIM, 3 * DIM], DT, kind="ExternalInput")
    w_ce = nc.dram_tensor("w_ce", [4, DIM, DIM], DT, kind="ExternalInput")
    w_f1 = nc.dram_tensor("w_f1", [DIM, DIM], DT, kind="ExternalInput")
    w_f2 = nc.dram_tensor("w_f2", [DIM, DIM], DT, kind="ExternalInput")
    w_f3 = nc.dram_tensor("w_f3", [DIM, DIM], DT, kind="ExternalInput")
    w_as1 = nc.dram_tensor("w_as1", [2 * DIM, 75], F32, kind="ExternalInput")
    w_as2 = nc.dram_tensor("w_as2", [75, 1], F32, kind="ExternalInput")
    # bias cols: 0 bz, 1 bo, 2..6 ce_b[0..4], 7 f1_b, 8 f2_b, 9 f3_b
    biases = nc.dram_tensor("biases", [DIM, 10], F32, kind="ExternalInput")
    b_as1 = nc.dram_tensor("b_as1", [75, 1], F32, kind="ExternalInput")
    scal = nc.dram_tensor("scal", [128, SC_NCOL], F32, kind="ExternalInput")
    scal_dt = nc.dram_tensor("scal_dt", [128, SC_NCOL], DT, kind="ExternalInput")
    out = nc.dram_tensor("scores", [BPC, 4], F32, kind="ExternalOutput")

    with tile.TileContext(nc) as tc:
        from contextlib import ExitStack
        with ExitStack() as ctx:
            _emit(nc, tc, ctx, emb, art_idx, q_idx, opt_idx, w_art, w_ce,
                  w_f1, w_f2, w_f3, w_as1, w_as2, biases, b_as1, scal,
                  scal_dt, out)

    nc.compile()
    return nc


def _emit(nc, tc, ctx, emb, art_idx, q_idx, opt_idx, w_art, w_ce, w_f1,
          w_f2, w_f3, w_as1, w_as2, biases, b_as1, scal, scal_dt, out):
    # ---------------- pools ----------------
    consts = ctx.enter_context(tc.tile_pool(name="consts", bufs=1))
    gpool = ctx.enter_context(tc.tile_pool(name="gather", bufs=6))
    p_art = ctx.enter_context(tc.tile_pool(name="p_art", bufs=2))
    p_enc = ctx.enter_context(tc.tile_pool(name="p_enc", bufs=2))
    p_zb = ctx.enter_context(tc.tile_pool(name="p_zb", bufs=1))
    p_mix = ctx.enter_context(tc.tile_pool(name="p_mix", bufs=2))
    p_xs = ctx.enter_context(tc.tile_pool(name="p_xs", bufs=1))
    small = ctx.enter_context(tc.tile_pool(name="small", bufs=3))
    pp500 = ctx.enter_context(tc.tile_pool(name="pp500", bufs=2, space="PSUM"))
    pptr = ctx.enter_context(tc.tile_pool(name="pptr", bufs=2, space="PSUM"))
    ppatt = ctx.enter_context(tc.tile_pool(name="ppatt", bufs=2, space="PSUM"))
    ppacc = ctx.enter_context(tc.tile_pool(name="ppacc", bufs=2, space="PSUM"))

    # ---------------- constants / weights ----------------
    ident = consts.tile([128, 128], DT)
    make_identity(nc, ident[:])

    w_art_sb = consts.tile([DCS, DC, 3 * DIM], DT)
    for kc in range(DC):
        nc.sync.dma_start(w_art_sb[:, kc, :], w_art[kc * DCS:(kc + 1) * DCS, :])
    w_ce_sb = consts.tile([DCS, DC, 4, DIM], DT)
    for kc in range(DC):
        for ri in range(4):
            nc.sync.dma_start(w_ce_sb[:, kc, ri, :],
                              w_ce[ri, kc * DCS:(kc + 1) * DCS, :])
    w_f1_sb = consts.tile([DCS, DC, DIM], DT)
    w_f2_sb = consts.tile([DCS, DC, DIM], DT)
    w_f3_sb = consts.tile([DCS, DC, DIM], DT)
    for w_sb, w_dram in ((w_f1_sb, w_f1), (w_f2_sb, w_f2), (w_f3_sb, w_f3)):
        for kc in range(DC):
            nc.sync.dma_start(w_sb[:, kc, :], w_dram[kc * DCS:(kc + 1) * DCS, :])
    w_as1_sb = consts.tile([DCS, 6, 75], F32)
    for j in range(6):
        nc.sync.dma_start(w_as1_sb[:, j, :], w_as1[j * DCS:(j + 1) * DCS, :])
    w_as2_sb = consts.tile([75, 1], F32)
    nc.sync.dma_start(w_as2_sb[:], w_as2[:])
    bias_sb = consts.tile([DCS, DC, 10], F32)
    for kc in range(DC):
        nc.sync.dma_start(bias_sb[:, kc, :], biases[kc * DCS:(kc + 1) * DCS, :])
    b_as1_sb = consts.tile([75, 1], F32)
    nc.sync.dma_start(b_as1_sb[:], b_as1[:])
    scal_sb = consts.tile([128, SC_NCOL], F32)
    nc.sync.dma_start(scal_sb[:], scal[:])
    scal_dt_sb = consts.tile([128, SC_NCOL], DT)
    nc.sync.dma_start(scal_dt_sb[:], scal_dt[:])

    def sc(col):  # f32 per-partition scalar [100,1]
        return scal_sb[0:DCS, col:col + 1]

    ans_sb = small.tile([DCS, BPC, 6, 4], F32, tag="ans_sb")

    for b in range(BPC):
        # ---------------- indices ----------------
        aidx = small.tile([128, NTCH], I32, tag="aidx")
        for c in range(NTCH):
            pc = TCH[c]
            nc.sync.dma_start(aidx[:pc, c:c + 1],
                              art_idx[b, c * 128:c * 128 + pc, None])
        qidx = small.tile([TQ, 1], I32, tag="qidx")
        nc.sync.dma_start(qidx[:], q_idx[b, :, None])
        oidx = small.tile([TO, 4], I32, tag="oidx")
        nc.sync.dma_start(oidx[:], opt_idx[b].rearrange("o w -> w o"))

        # ---------------- gathers + transposes ----------------
        artT = [p_art.tile([DCS, T], DT, tag=f"artT{dc}", name=f"artT{dc}") for dc in range(DC)]
        for c in range(NTCH):
            pc = TCH[c]
            g = gpool.tile([128, DIM], DT, tag="gart")
            nc.gpsimd.indirect_dma_start(
                out=g[:pc, :], out_offset=None, in_=emb[:],
                in_offset=bass.IndirectOffsetOnAxis(ap=aidx[:pc, c:c + 1], axis=0))
            for dc in range(DC):
                tp = pptr.tile([DCS, 128], DT, tag="tr")
                nc.tensor.transpose(tp[:, :pc], g[:pc, dc * DCS:(dc + 1) * DCS],
                                    ident[:pc, :pc])
                nc.vector.tensor_copy(artT[dc][:, c * 128:c * 128 + pc],
                                      tp[:, :pc])

        qg = small.tile([TQ, DIM], DT, tag="qg")
        nc.gpsimd.indirect_dma_start(
            out=qg[:], out_offset=None, in_=emb[:],
            in_offset=bass.IndirectOffsetOnAxis(ap=qidx[:, 0:1], axis=0))
        qT = small.tile([DCS, DC, TQ], DT, tag="qT")
        for dc in range(DC):
            tp = pptr.tile([DCS, 128], DT, tag="tr")
            nc.tensor.transpose(tp[:, :TQ], qg[:, dc * DCS:(dc + 1) * DCS],
                                ident[:TQ, :TQ])
            nc.vector.tensor_copy(qT[:, dc, :], tp[:, :TQ])

        og = [small.tile([TO, DIM], DT, tag=f"og{o}", name=f"og{o}") for o in range(4)]
        oT = small.tile([DCS, DC, 4, TO], DT, tag="oT")
        for o in range(4):
            nc.gpsimd.indirect_dma_start(
                out=og[o][:], out_offset=None, in_=emb[:],
                in_offset=bass.IndirectOffsetOnAxis(ap=oidx[:, o:o + 1], axis=0))
            for dc in range(DC):
                tp = pptr.tile([DCS, 128], DT, tag="tr")
                nc.tensor.transpose(tp[:, :TO], og[o][:, dc * DCS:(dc + 1) * DCS],
                                    ident[:TO, :TO])
                nc.vector.tensor_copy(oT[:, dc, o, :], tp[:, :TO])

        # ---------------- group sums (xs_r in [d, g] layout) ----------------
        xs2 = [p_xs.tile([DCS, T // 2], DT, tag=f"xs2_{dc}", name=f"xs2_{dc}") for dc in range(DC)]
        xs4 = [p_xs.tile([DCS, T // 4], DT, tag=f"xs4_{dc}", name=f"xs4_{dc}") for dc in range(DC)]
        xs10 = [p_xs.tile([DCS, T // 10], DT, tag=f"xs10_{dc}", name=f"xs10_{dc}") for dc in range(DC)]
        xs25 = [p_xs.tile([DCS, T // 25], DT, tag=f"xs25_{dc}", name=f"xs25_{dc}") for dc in range(DC)]
        for dc in range(DC):
            a = artT[dc]
            nc.gpsimd.tensor_add(xs2[dc][:], a[:, 0:T:2], a[:, 1:T:2])
            nc.gpsimd.tensor_add(xs4[dc][:], xs2[dc][:, 0:T // 2:2],
                                 xs2[dc][:, 1:T // 2:2])
            with nc.allow_low_precision(reason="bf16 group sums feed bf16 matmuls"):
                nc.vector.tensor_reduce(
                    xs10[dc][:], xs2[dc][:].rearrange("p (g r) -> p g r", r=5),
                    AX.X, OP.add)
                nc.vector.tensor_reduce(
                    xs25[dc][:], a[:].rearrange("p (g r) -> p g r", r=25),
                    AX.X, OP.add)

        # ---------------- z / o / B1 (art stream) ----------------
        z_sb = [p_zb.tile([DCS, T], DT, tag=f"z{dc}", name=f"z{dc}") for dc in range(DC)]
        o_sb = [p_enc.tile([DCS, T], DT, tag=f"o{dc}", name=f"o{dc}") for dc in range(DC)]
        b1_sb = [p_zb.tile([DCS, T], DT, tag=f"b1_{dc}", name=f"b1_{dc}") for dc in range(DC)]
        for mi, (dst, func, bcol) in enumerate(
                ((z_sb, AF.Tanh, 0), (o_sb, AF.Tanh, 1), (b1_sb, AF.Relu, 2))):
            for dc in range(DC):
                mcol = mi * DIM + dc * DCS
                for t0 in range(0, T, N_MM):
                    ps = pp500.tile([DCS, N_MM], F32, tag="mm")
                    for kc in range(DC):
                        nc.tensor.matmul(
                            ps[:], w_art_sb[:, kc, mcol:mcol + DCS],
                            artT[kc][:, t0:t0 + N_MM],
                            start=(kc == 0), stop=(kc == DC - 1))
                    nc.scalar.activation(dst[dc][:, t0:t0 + N_MM], ps[:],
                                         func, bias=bias_sb[:, dc, bcol:bcol + 1])

        # ---------------- CE r>=2 ----------------
        bl = {}
        for ri, (xs, r) in enumerate(((xs2, 2), (xs4, 4), (xs10, 10), (xs25, 25))):
            g_r = T // r
            bl[r] = [p_xs.tile([DCS, g_r], DT, tag=f"bl{r}_{dc}", name=f"bl{r}_{dc}")
                     for dc in range(DC)]
            for dc in range(DC):
                for g0 in range(0, g_r, N_MM):
                    gn = min(N_MM, g_r - g0)
                    ps = pp500.tile([DCS, N_MM], F32, tag="mm")
                    for kc in range(DC):
                        nc.tensor.matmul(
                            ps[:, :gn],
                            w_ce_sb[:, kc, ri, dc * DCS:(dc + 1) * DCS],
                            xs[kc][:, g0:g0 + gn],
                            start=(kc == 0), stop=(kc == DC - 1))
                    nc.scalar.activation(bl[r][dc][:, g0:g0 + gn], ps[:, :gn],
                                         AF.Relu, bias=bias_sb[:, dc, 3 + ri:4 + ri])

        # ---------------- gate mix ----------------
        gate = []
        for dc in range(DC):
            ev = [b1_sb[dc][:]]
            for r in (2, 4, 10, 25):
                ev.append(bl[r][dc][:, :, None].to_broadcast([DCS, T // r, r]))
            h1 = []
            for k in range(3):
                eng = nc.vector  # scalar_tensor_tensor is DVE-only
                acc = p_mix.tile([DCS, T], DT, tag=f"h1_{k}")
                eng.scalar_tensor_tensor(
                    acc[:], ev[0], sc(SC_M1 + 5 * k),
                    scal_dt_sb[0:DCS, SC_M1B + k:SC_M1B + k + 1]
                    .to_broadcast([DCS, T]),
                    op0=OP.mult, op1=OP.add)
                for ri in range(1, 5):
                    eng.scalar_tensor_tensor(
                        acc[:], ev[ri], sc(SC_M1 + 5 * k + ri), acc[:],
                        op0=OP.mult, op1=OP.add)
                nc.scalar.activation(acc[:], acc[:], AF.Relu)
                h1.append(acc)
            g_acc = p_mix.tile([DCS, T], DT, tag="gate")
            nc.vector.scalar_tensor_tensor(
                g_acc[:], h1[0][:], sc(SC_M2),
                scal_dt_sb[0:DCS, SC_M2B:SC_M2B + 1].to_broadcast([DCS, T]),
                op0=OP.mult, op1=OP.add)
            nc.vector.scalar_tensor_tensor(
                g_acc[:], h1[1][:], sc(SC_M2 + 1), g_acc[:],
                op0=OP.mult, op1=OP.add)
            nc.vector.scalar_tensor_tensor(
                g_acc[:], h1[2][:], sc(SC_M2 + 2), g_acc[:],
                op0=OP.mult, op1=OP.add)
            nc.scalar.activation(g_acc[:], g_acc[:], AF.Relu)
            gate.append(g_acc)

        # ---------------- MRU scan + encode ----------------
        encT = []
        for dc in range(DC):
            gz = p_mix.tile([DCS, T], DT, tag="gz")
            nc.gpsimd.tensor_tensor(gz[:], gate[dc][:], z_sb[dc][:], op=OP.mult)
            nc.vector.tensor_sub(z_sb[dc][:], z_sb[dc][:], gz[:])  # (1-g)z
            c_t = p_mix.tile([DCS, T], DT, tag="c")
            nc.vector.tensor_tensor_scan(
                c_t[:], gate[dc][:], z_sb[dc][:], 0.0, op0=OP.mult, op1=OP.add)
            nc.vector.tensor_mul(o_sb[dc][:], o_sb[dc][:], c_t[:])
            encT.append(o_sb[dc])

        # ---------------- keys1T ----------------
        k1T = small.tile([DCS, DC, TQ], DT, tag="k1T")
        for dc in range(DC):
            ps = ppatt.tile([DCS, TQ], F32, tag="att")
            for kc in range(DC):
                nc.tensor.matmul(ps[:], w_f1_sb[:, kc, dc * DCS:(dc + 1) * DCS],
                                 qT[:, kc, :], start=(kc == 0), stop=(kc == DC - 1))
            nc.scalar.copy(k1T[:, dc, :], ps[:])

        # ---------------- A2/A3 and QK ----------------
        aTs = []
        for fi, w_f_sb in enumerate((w_f2_sb, w_f3_sb)):
            a_ps = ppatt.tile([TQ, DIM], F32, tag="att")
            for kc in range(DC):
                nc.tensor.matmul(a_ps[:], qT[:, kc, :], w_f_sb[:, kc, :],
                                 start=(kc == 0), stop=(kc == DC - 1))
            a_sb = small.tile([TQ, DIM], DT, tag="a_sb")
            nc.vector.tensor_copy(a_sb[:], a_ps[:])
            aT = small.tile([DCS, DC, TQ], DT, tag=f"aT{fi}")
            for dc in range(DC):
                tp = pptr.tile([DCS, 128], DT, tag="tr")
                nc.tensor.transpose(tp[:, :TQ], a_sb[:, dc * DCS:(dc + 1) * DCS],
                                    ident[:TQ, :TQ])
                nc.vector.tensor_copy(aT[:, dc, :], tp[:, :TQ])
            aTs.append(aT)

        qk_ps = ppacc.tile([TQ, 128], F32, tag="acc")
        for fi in range(2):
            for o in range(4):
                gcol = 16 * (4 * fi + o)
                for kc in range(DC):
                    nc.tensor.matmul(qk_ps[:, gcol:gcol + 16],
                                     aTs[fi][:, kc, :], oT[:, kc, o, :],
                                     start=(kc == 0), stop=(kc == DC - 1))
        qk_sb = small.tile([TQ, 128], DT, tag="qk_sb")
        nc.vector.tensor_copy(qk_sb[:], qk_ps[:])

        # ---------------- attention stream over t chunks ----------------
        pb_ps = ppacc.tile([128, 8], F32, tag="acc")
        for c in range(NTCH):
            pc = TCH[c]
            s1 = ppatt.tile([128, TQ], F32, tag="att")
            for dc in range(DC):
                nc.tensor.matmul(s1[:pc, :], encT[dc][:, c * 128:c * 128 + pc],
                                 k1T[:, dc, :], start=(dc == 0),
                                 stop=(dc == DC - 1))
            e1 = small.tile([128, TQ], DT, tag="e1")
            z1 = small.tile([128, 2], F32, tag="z1")
            nc.scalar.activation(e1[:pc, :], s1[:pc, :], AF.Exp,
                                 accum_out=z1[:pc, 0:1])
            nc.vector.reciprocal(z1[:pc, 1:2], z1[:pc, 0:1])
            tp = pptr.tile([TQ, 128], DT, tag="tr")
            nc.tensor.transpose(tp[:, :pc], e1[:pc, :], ident[:pc, :pc])
            e1T = small.tile([TQ, 128], DT, tag="e1Ts")
            nc.vector.tensor_copy(e1T[:, :pc], tp[:, :pc])
            u2 = ppatt.tile([128, 128], F32, tag="att")
            nc.tensor.matmul(u2[:pc, :], e1T[:, :pc], qk_sb[:],
                             start=True, stop=True)
            e2 = small.tile([128, 128], DT, tag="e2")
            nc.scalar.activation(e2[:pc, :], u2[:pc, :], AF.Exp,
                                 scale=z1[:pc, 1:2])
            z2 = small.tile([128, 16], F32, tag="z2")
            nc.vector.tensor_reduce(z2[:pc, 0:8],
                                    e2[:pc, :].rearrange("p (g w) -> p g w", w=16),
                                    AX.X, OP.add)
            nc.vector.reciprocal(z2[:pc, 8:16], z2[:pc, 0:8])
            rz2 = small.tile([128, 8], DT, tag="rz2")
            # fold the mean-over-t (1/T) into the reduction weights
            nc.vector.tensor_scalar_mul(rz2[:pc, :], z2[:pc, 8:16], 1.0 / T)
            nc.tensor.matmul(pb_ps[:, :], e2[:pc, :], rz2[:pc, :],
                             start=(c == 0), stop=(c == NTCH - 1))

        # ---------------- answer vectors ----------------
        pb_sb = small.tile([128, 8], DT, tag="pb_sb")
        nc.vector.tensor_copy(pb_sb[:], pb_ps[:])
        ans_ps = ppacc.tile([DCS, 24], F32, tag="acc")
        for g in range(8):
            fi, o = g // 4, g % 4
            pb16 = small.tile([TO, 1], DT, tag="pb16")
            nc.sync.dma_start(pb16[:], pb_sb[16 * g:16 * g + 16, g:g + 1])
            for dc in range(DC):
                j = fi * 3 + dc
                nc.tensor.matmul(ans_ps[:, j * 4 + o:j * 4 + o + 1],
                                 og[o][:, dc * DCS:(dc + 1) * DCS], pb16[:],
                                 start=True, stop=True)
        nc.vector.tensor_copy(ans_sb[:, b, :, :].rearrange("p j o -> p (j o)"),
                              ans_ps[:])

    # ---------------- final MLP (both batches together) ----------------
    h_ps = ppatt.tile([75, 8], F32, tag="att")
    for j in range(6):
        # rhs columns = (b, o) pairs for chunk j of the 600-dim ans vector
        rhs = ans_sb[:, :, j, :]
        nc.tensor.matmul(h_ps[:], w_as1_sb[:, j, :], rhs,
                         start=(j == 0), stop=(j == 5))
    h_sb = small.tile([75, 8], F32, tag="h_sb")
    nc.scalar.activation(h_sb[:], h_ps[:], AF.Relu, bias=b_as1_sb[:])
    s_ps = ppacc.tile([8, 1], F32, tag="acc")
    nc.tensor.matmul(s_ps[:], h_sb[:], w_as2_sb[:], start=True, stop=True)
    s_sb = small.tile([8, 1], F32, tag="s_sb")
    nc.scalar.activation(s_sb[:], s_ps[:], AF.Identity,
                         bias=scal_sb[0:8, SC_AS2B:SC_AS2B + 1])
    nc.sync.dma_start(out[:].rearrange("b o -> (b o)")[:, None], s_sb[:])


# ---------------------------------------------------------------------------
# host side
# ---------------------------------------------------------------------------

_CACHE = {}


def _get_nc():
    if "nc" not in _CACHE:
        _CACHE["nc"] = _build_program()
    return _CACHE["nc"]


def _prep_core_inputs(inputs, core):
    b0 = core * BPC
    sl = slice(b0, b0 + BPC)
    f = np.asarray
    prep = _CACHE.get("prep_shared")
    if prep is None:
        # core-independent tensors, computed once per kernel() call set
        Wz, Wo = f(inputs["Wz"]), f(inputs["Wo"])
        ceW = f(inputs["ce_W"])
        prep = {
            "emb": f(inputs["emb"]).astype(NPDT),
            "w_art": np.ascontiguousarray(
                np.concatenate([Wz.T, Wo.T, ceW[0].T], axis=1)).astype(NPDT),
            "w_ce": np.ascontiguousarray(
                ceW[1:].transpose(0, 2, 1)).astype(NPDT),
            "w_f1": np.ascontiguousarray(f(inputs["f1_W"]).T).astype(NPDT),
            # s2 = aoq @ f2W @ opt^T, so f2/f3 go in UNtransposed
            # (f1 builds keys1^T = f1W @ q^T and does need the transpose)
            "w_f2": np.ascontiguousarray(f(inputs["f2_W"])).astype(NPDT),
            "w_f3": np.ascontiguousarray(f(inputs["f3_W"])).astype(NPDT),
            "w_as1": np.ascontiguousarray(f(inputs["as1_W"]).T).astype(np.float32),
            "w_as2": np.ascontiguousarray(f(inputs["as2_W"]).T).astype(np.float32),
            "biases": np.stack(
                [f(inputs["bz"]), f(inputs["bo"]),
                 *[f(inputs["ce_b"])[i] for i in range(5)],
                 f(inputs["f1_b"]), f(inputs["f2_b"]), f(inputs["f3_b"])],
                axis=1).astype(np.float32),
            "b_as1": f(inputs["as1_b"])[:, None].astype(np.float32),
        }
        scal = np.zeros((128, SC_NCOL), np.float32)
        m1 = f(inputs["mr1_W"])
        for k in range(3):
            for ri, r in enumerate(RANGES):
                scal[:, SC_M1 + 5 * k + ri] = m1[k, ri] / r
        scal[:, SC_M1B:SC_M1B + 3] = f(inputs["mr1_b"])[None, :]
        scal[:, SC_M2:SC_M2 + 3] = f(inputs["mr2_W"])[0][None, :]
        scal[:, SC_M2B] = f(inputs["mr2_b"])[0]
        scal[:, SC_AS2B] = f(inputs["as2_b"])[0]
        prep["scal"] = scal
        prep["scal_dt"] = scal.astype(NPDT)
        _CACHE["prep_shared"] = prep

    d = dict(prep)
    d["art_idx"] = f(inputs["article_in"])[sl].astype(np.int32)
    d["q_idx"] = f(inputs["question_in"])[sl].astype(np.int32)
    d["opt_idx"] = np.stack(
        [f(inputs[f"option{i}_in"])[sl] for i in (1, 2, 3, 4)],
        axis=1).astype(np.int32)
    return d


def _get_runner():
    """jit-compiled 8-core runner, built once per process."""
    if "runner" in _CACHE:
        return _CACHE["runner"]
    import jax
    from jax.sharding import Mesh, PartitionSpec
    from jax.experimental.shard_map import shard_map
    from concourse.bass2jax import (_bass_exec_p, install_neuronx_cc_hook,
                                    partition_id_tensor)

    install_neuronx_cc_hook()
    nc = _get_nc()
    pid_name = nc.partition_id_tensor.name if nc.partition_id_tensor else None

    in_names, out_names, out_avals, zero_outs = [], [], [], []
    for alloc in nc.m.functions[0].allocations:
        if not isinstance(alloc, mybir.MemoryLocationSet):
            continue
        name = alloc.memorylocations[0].name
        if alloc.kind == "ExternalInput":
            if name != pid_name:
                in_names.append(name)
        elif alloc.kind == "ExternalOutput":
            out_names.append(name)
            shape = tuple(alloc.tensor_shape)
            dtype = mybir.dt.np(alloc.dtype)
            out_avals.append(jax.core.ShapedArray(shape, dtype))
            zero_outs.append(np.zeros(shape, dtype))
    n_params = len(in_names)
    all_in_names = in_names + out_names
    if pid_name is not None:
        all_in_names = all_in_names + [pid_name]

    def _body(*args):
        operands = list(args)
        if pid_name is not None:
            operands.append(partition_id_tensor())
        outs = _bass_exec_p.bind(
            *operands, out_avals=tuple(out_avals), in_names=tuple(all_in_names),
            out_names=tuple(out_names), lowering_input_output_aliases=(),
            sim_require_finite=True, sim_require_nnan=True, nc=nc)
        return tuple(outs)

    devices = jax.devices()[:NCORES]
    mesh = Mesh(np.asarray(devices), ("core",))
    in_specs = (PartitionSpec("core"),) * (n_params + len(out_names))
    out_specs = (PartitionSpec("core"),) * len(out_names)
    sharded = jax.jit(shard_map(_body, mesh=mesh, in_specs=in_specs,
                                out_specs=out_specs, check_rep=False),
                      keep_unused=True)

    _CACHE["runner"] = (sharded, in_names, out_names, zero_outs)
    return _CACHE["runner"]


def run_cores(per_core_inputs):
    """per_core_inputs: list of 8 dicts name->np array. Returns out dicts."""
    sharded, in_names, out_names, zero_outs = _get_runner()
    concat_in = [np.concatenate([per_core_inputs[c][n] for c in range(NCORES)],
                                axis=0) for n in in_names]
    concat_zeros = [np.concatenate([z] * NCORES, axis=0) for z in zero_outs]
    outs = sharded(*concat_in, *concat_zeros)
    result = []
    for c in range(NCORES):
        d = {}
        for i, n in enumerate(out_names):
            arr = np.asarray(outs[i])
            per = arr.shape[0] // NCORES
            d[n] = arr[c * per:(c + 1) * per]
        result.append(d)
    return result


def prepare_device_args(per_core_inputs):
    """device_put the concatenated inputs once, for repeated timed runs."""
    import jax
    from jax.sharding import Mesh, PartitionSpec, NamedSharding
    sharded, in_names, out_names, zero_outs = _get_runner()
    devices = jax.devices()[:NCORES]
    mesh = Mesh(np.asarray(devices), ("core",))
    sh = NamedSharding(mesh, PartitionSpec("core"))
    concat_in = [np.concatenate([per_core_inputs[c][n] for c in range(NCORES)],
                                axis=0) for n in in_names]
    concat_zeros = [np.concatenate([z] * NCORES, axis=0) for z in zero_outs]
    args = [jax.device_put(a, sh) for a in concat_in + concat_zeros]
    jax.block_until_ready(args)
    return args


def run_prepared(dev_args):
    sharded, in_names, out_names, zero_outs = _get_runner()
    outs = sharded(*dev_args)
    import jax
    jax.block_until_ready(outs)
    return outs


def kernel(**inputs):
    _CACHE.pop("prep_shared", None)
    per_core = [_prep_core_inputs(inputs, c) for c in range(NCORES)]
    res = run_cores(per_core)
    out = np.concatenate([res[c]["scores"] for c in range(NCORES)], axis=0)
    return out.astype(np.float32)


# revision 21
# speedup vs baseline: 136.4704x; 1.0131x over previous
"""BiAttentionMRU Trainium2 kernel.

Data-parallel over batch: B=16 -> 2 batch elements on each of 8 cores.
All weights replicated. Embedding gather done on-device via indirect DMA.

Layouts (per core, per batch element b in {0,1}):
  art gathered as [t-chunk(128), d=300], PE-transposed into artT[dc][100, 2000]
  (d on partitions, 3 chunks of 100). Group sums, z/o/CE matmuls, gate mix,
  MRU scan (native tensor_tensor_scan along t) and the attention lhsT all
  work in [d, t] layout.

Attention algebra: aoq is never materialized. With e1 = exp(art_enc @ keys1^T),
Z1 its row sum, s2 = softmax-normalized p1 @ (q @ keys_f^T) is computed as
exp-of(u2 * 1/Z1) where u2 = e1 @ QK. The per-option mean over t of
softmax(s2) @ opt folds into one accumulating matmul sum_t e2[t,:] * (1/Z2[t]).
"""

import sys

sys.path.insert(0, "/opt/trn_rl_repo")

import numpy as np
import ml_dtypes

import concourse.bass as bass
import concourse.tile as tile
from concourse import bacc, mybir
from concourse.masks import make_identity

F32 = mybir.dt.float32
BF16 = mybir.dt.bfloat16
I32 = mybir.dt.int32
AX = mybir.AxisListType
OP = mybir.AluOpType
AF = mybir.ActivationFunctionType

DIM = 300
VOCAB = 50000
B_FULL = 16
NCORES = 8
BPC = B_FULL // NCORES  # batch per core = 2
T = 2000
TQ = 30
TO = 16
RANGES = (1, 2, 4, 10, 25)

TCH = [128] * 15 + [80]  # t chunking for transposes / attention
NTCH = len(TCH)
DC = 3  # d chunks of 100
DCS = 100

N_MM = 500  # matmul N-chunking for [d,t] streams (psum free <= 512 fp32)

USE_BF16 = True
DT = BF16 if USE_BF16 else F32
NPDT = ml_dtypes.bfloat16 if USE_BF16 else np.float32

# scalar table columns (host-packed, replicated down 128 partitions)
SC_M1 = 0      # 15 cols: m1[k,r]/r at 5k+ri
SC_M1B = 15    # 3 cols
SC_M2 = 18     # 3 cols
SC_M2B = 21    # 1 col
SC_AS2B = 22   # 1 col
SC_NCOL = 24


def _build_program():
    nc = bacc.Bacc("TRN2", target_bir_lowering=False, debug=False,
                   num_devices=NCORES)

    emb = nc.dram_tensor("emb", [VOCAB, DIM], DT, kind="ExternalInput")
    art_idx = nc.dram_tensor("art_idx", [BPC, T], I32, kind="ExternalInput")
    q_idx = nc.dram_tensor("q_idx", [BPC, TQ], I32, kind="ExternalInput")
    opt_idx = nc.dram_tensor("opt_idx", [BPC, 4, TO], I32, kind="ExternalInput")
    w_art = nc.dram_tensor("w_art", [DIM, 3 * DIM], DT, kind="ExternalInput")
    w_ce = nc.dram_tensor("w_ce", [4, DIM, DIM], DT, kind="ExternalInput")
    w_f1 = nc.dram_tensor("w_f1", [DIM, DIM], DT, kind="ExternalInput")
    w_f2 = nc.dram_tensor("w_f2", [DIM, DIM], DT, kind="ExternalInput")
    w_f3 = nc.dram_tensor("w_f3", [DIM, DIM], DT, kind="ExternalInput")
    w_as1 = nc.dram_tensor("w_as1", [2 * DIM, 75], F32, kind="ExternalInput")
    w_as2 = nc.dram_tensor("w_as2", [75, 1], F32, kind="ExternalInput")
    # bias cols: 0 bz, 1 bo, 2..6 ce_b[0..4], 7 f1_b, 8 f2_b, 9 f3_b
    biases = nc.dram_tensor("biases", [DIM, 10], F32, kind="ExternalInput")
    b_as1 = nc.dram_tensor("b_as1", [75, 1], F32, kind="ExternalInput")
    scal = nc.dram_tensor("scal", [128, SC_NCOL], F32, kind="ExternalInput")
    scal_dt = nc.dram_tensor("scal_dt", [128, SC_NCOL], DT, kind="ExternalInput")
    out = nc.dram_tensor("scores", [BPC, 4], F32, kind="ExternalOutput")

    with tile.TileContext(nc) as tc:
        from contextlib import ExitStack
        with ExitStack() as ctx:
            _emit(nc, tc, ctx, emb, art_idx, q_idx, opt_idx, w_art, w_ce,
                  w_f1, w_f2, w_f3, w_as1, w_as2, biases, b_as1, scal,
                  scal_dt, out)

    nc.compile()
    return nc


def _emit(nc, tc, ctx, emb, art_idx, q_idx, opt_idx, w_art, w_ce, w_f1,
          w_f2, w_f3, w_as1, w_as2, biases, b_as1, scal, scal_dt, out):
    # ---------------- pools ----------------
    consts = ctx.enter_context(tc.tile_pool(name="consts", bufs=1))
    gpool = ctx.enter_context(tc.tile_pool(name="gather", bufs=6))
    p_art = ctx.enter_context(tc.tile_pool(name="p_art", bufs=2))
    p_enc = ctx.enter_context(tc.tile_pool(name="p_enc", bufs=2))
    p_zb = ctx.enter_context(tc.tile_pool(name="p_zb", bufs=1))
    p_mix = ctx.enter_context(tc.tile_pool(name="p_mix", bufs=2))
    p_xs = ctx.enter_context(tc.tile_pool(name="p_xs", bufs=1))
    small = ctx.enter_context(tc.tile_pool(name="small", bufs=4))
    pp500 = ctx.enter_context(tc.tile_pool(name="pp500", bufs=2, space="PSUM"))
    ppwork = ctx.enter_context(tc.tile_pool(name="ppwork", bufs=4, space="PSUM"))
    pptr = ppwork
    ppatt = ppwork
    ppacc = ctx.enter_context(tc.tile_pool(name="ppacc", bufs=2, space="PSUM"))

    # ---------------- constants / weights ----------------
    ident = consts.tile([128, 128], DT)
    make_identity(nc, ident[:])

    w_art_sb = consts.tile([DCS, DC, 3 * DIM], DT)
    for kc in range(DC):
        nc.sync.dma_start(w_art_sb[:, kc, :], w_art[kc * DCS:(kc + 1) * DCS, :])
    w_ce_sb = consts.tile([DCS, DC, 4, DIM], DT)
    for kc in range(DC):
        for ri in range(4):
            nc.sync.dma_start(w_ce_sb[:, kc, ri, :],
                              w_ce[ri, kc * DCS:(kc + 1) * DCS, :])
    w_f1_sb = consts.tile([DCS, DC, DIM], DT)
    w_f2_sb = consts.tile([DCS, DC, DIM], DT)
    w_f3_sb = consts.tile([DCS, DC, DIM], DT)
    for w_sb, w_dram in ((w_f1_sb, w_f1), (w_f2_sb, w_f2), (w_f3_sb, w_f3)):
        for kc in range(DC):
            nc.sync.dma_start(w_sb[:, kc, :], w_dram[kc * DCS:(kc + 1) * DCS, :])
    w_as1_sb = consts.tile([DCS, 6, 75], F32)
    for j in range(6):
        nc.sync.dma_start(w_as1_sb[:, j, :], w_as1[j * DCS:(j + 1) * DCS, :])
    w_as2_sb = consts.tile([75, 1], F32)
    nc.sync.dma_start(w_as2_sb[:], w_as2[:])
    bias_sb = consts.tile([DCS, DC, 10], F32)
    for kc in range(DC):
        nc.sync.dma_start(bias_sb[:, kc, :], biases[kc * DCS:(kc + 1) * DCS, :])
    b_as1_sb = consts.tile([75, 1], F32)
    nc.sync.dma_start(b_as1_sb[:], b_as1[:])
    scal_sb = consts.tile([128, SC_NCOL], F32)
    nc.sync.dma_start(scal_sb[:], scal[:])
    scal_dt_sb = consts.tile([128, SC_NCOL], DT)
    nc.sync.dma_start(scal_dt_sb[:], scal_dt[:])

    def sc(col):  # f32 per-partition scalar [100,1]
        return scal_sb[0:DCS, col:col + 1]

    # scaled 100x100 identities for the PE-side gate mix:
    # cols j=5k+ri hold m1[k,ri]/r * I, cols 15+k hold m2[k] * I
    mI = consts.tile([DCS, 18, DCS], DT)
    for j in range(18):
        scol = (SC_M1 + j) if j < 15 else (SC_M2 + j - 15)
        nc.vector.tensor_scalar_mul(mI[:, j, :], ident[0:DCS, 0:DCS], sc(scol))

    ans_sb = small.tile([DCS, BPC, 6, 4], F32, tag="ans_sb")

    gathered = []
    for b in range(BPC):
        # ---------------- indices ----------------
        aidx = small.tile([128, NTCH], I32, tag="aidx")
        for c in range(NTCH):
            pc = TCH[c]
            nc.sync.dma_start(aidx[:pc, c:c + 1],
                              art_idx[b, c * 128:c * 128 + pc, None])
        qidx = small.tile([TQ, 1], I32, tag="qidx")
        nc.sync.dma_start(qidx[:], q_idx[b, :, None])
        oidx = small.tile([TO, 4], I32, tag="oidx")
        nc.sync.dma_start(oidx[:], opt_idx[b].rearrange("o w -> w o"))

        # ---------------- gathers + transposes ----------------
        artT = [p_art.tile([DCS, T], DT, tag=f"artT{dc}", name=f"artT{dc}") for dc in range(DC)]
        for c in range(NTCH):
            pc = TCH[c]
            g = gpool.tile([128, DIM], DT, tag="gart")
            nc.gpsimd.indirect_dma_start(
                out=g[:pc, :], out_offset=None, in_=emb[:],
                in_offset=bass.IndirectOffsetOnAxis(ap=aidx[:pc, c:c + 1], axis=0))
            for dc in range(DC):
                tp = pptr.tile([DCS, 128], DT, tag="work")
                nc.tensor.transpose(tp[:, :pc], g[:pc, dc * DCS:(dc + 1) * DCS],
                                    ident[:pc, :pc])
                nc.vector.tensor_copy(artT[dc][:, c * 128:c * 128 + pc],
                                      tp[:, :pc])

        qg = small.tile([TQ, DIM], DT, tag="qg")
        nc.gpsimd.indirect_dma_start(
            out=qg[:], out_offset=None, in_=emb[:],
            in_offset=bass.IndirectOffsetOnAxis(ap=qidx[:, 0:1], axis=0))
        qT = small.tile([DCS, DC, TQ], DT, tag="qT")
        for dc in range(DC):
            tp = pptr.tile([DCS, 128], DT, tag="work")
            nc.tensor.transpose(tp[:, :TQ], qg[:, dc * DCS:(dc + 1) * DCS],
                                ident[:TQ, :TQ])
            nc.vector.tensor_copy(qT[:, dc, :], tp[:, :TQ])

        og = [small.tile([TO, DIM], DT, tag=f"og{o}", name=f"og{o}") for o in range(4)]
        oT = small.tile([DCS, DC, 4, TO], DT, tag="oT")
        for o in range(4):
            nc.gpsimd.indirect_dma_start(
                out=og[o][:], out_offset=None, in_=emb[:],
                in_offset=bass.IndirectOffsetOnAxis(ap=oidx[:, o:o + 1], axis=0))
            for dc in range(DC):
                tp = pptr.tile([DCS, 128], DT, tag="work")
                nc.tensor.transpose(tp[:, :TO], og[o][:, dc * DCS:(dc + 1) * DCS],
                                    ident[:TO, :TO])
                nc.vector.tensor_copy(oT[:, dc, o, :], tp[:, :TO])

        gathered.append(dict(artT=artT, qg=qg, qT=qT, og=og, oT=oT))

    for b in range(BPC):
        artT = gathered[b]["artT"]
        qg = gathered[b]["qg"]
        qT = gathered[b]["qT"]
        og = gathered[b]["og"]
        oT = gathered[b]["oT"]

        # ---------------- group sums (xs_r in [d, g] layout) ----------------
        xs2 = [p_xs.tile([DCS, T // 2], DT, tag=f"xs2_{dc}", name=f"xs2_{dc}") for dc in range(DC)]
        xs4 = [p_xs.tile([DCS, T // 4], DT, tag=f"xs4_{dc}", name=f"xs4_{dc}") for dc in range(DC)]
        xs10 = [p_xs.tile([DCS, T // 10], DT, tag=f"xs10_{dc}", name=f"xs10_{dc}") for dc in range(DC)]
        xs25 = [p_xs.tile([DCS, T // 25], DT, tag=f"xs25_{dc}", name=f"xs25_{dc}") for dc in range(DC)]
        for dc in range(DC):
            a = artT[dc]
            nc.gpsimd.tensor_add(xs2[dc][:], a[:, 0:T:2], a[:, 1:T:2])
            nc.gpsimd.tensor_add(xs4[dc][:], xs2[dc][:, 0:T // 2:2],
                                 xs2[dc][:, 1:T // 2:2])
            with nc.allow_low_precision(reason="bf16 group sums feed bf16 matmuls"):
                nc.vector.tensor_reduce(
                    xs10[dc][:], xs2[dc][:].rearrange("p (g r) -> p g r", r=5),
                    AX.X, OP.add)
                nc.vector.tensor_reduce(
                    xs25[dc][:], a[:].rearrange("p (g r) -> p g r", r=25),
                    AX.X, OP.add)

        # ---------------- z / o / B1 (art stream) ----------------
        z_sb = [p_zb.tile([DCS, T], DT, tag=f"z{dc}", name=f"z{dc}") for dc in range(DC)]
        o_sb = [p_enc.tile([DCS, T], DT, tag=f"o{dc}", name=f"o{dc}") for dc in range(DC)]
        b1_sb = [p_zb.tile([DCS, T], DT, tag=f"b1_{dc}", name=f"b1_{dc}") for dc in range(DC)]
        for mi, (dst, func, bcol) in enumerate(
                ((z_sb, AF.Tanh, 0), (o_sb, AF.Tanh, 1), (b1_sb, AF.Relu, 2))):
            for dc in range(DC):
                mcol = mi * DIM + dc * DCS
                for t0 in range(0, T, N_MM):
                    ps = pp500.tile([DCS, N_MM], F32, tag="mm")
                    for kc in range(DC):
                        nc.tensor.matmul(
                            ps[:], w_art_sb[:, kc, mcol:mcol + DCS],
                            artT[kc][:, t0:t0 + N_MM],
                            start=(kc == 0), stop=(kc == DC - 1))
                    nc.scalar.activation(dst[dc][:, t0:t0 + N_MM], ps[:],
                                         func, bias=bias_sb[:, dc, bcol:bcol + 1])

        # ---------------- CE r>=2 ----------------
        bl = {}
        for ri, (xs, r) in enumerate(((xs2, 2), (xs4, 4), (xs10, 10), (xs25, 25))):
            g_r = T // r
            bl[r] = [p_xs.tile([DCS, g_r], DT, tag=f"bl{r}_{dc}", name=f"bl{r}_{dc}")
                     for dc in range(DC)]
            for dc in range(DC):
                for g0 in range(0, g_r, N_MM):
                    gn = min(N_MM, g_r - g0)
                    ps = pp500.tile([DCS, N_MM], F32, tag="mm")
                    for kc in range(DC):
                        nc.tensor.matmul(
                            ps[:, :gn],
                            w_ce_sb[:, kc, ri, dc * DCS:(dc + 1) * DCS],
                            xs[kc][:, g0:g0 + gn],
                            start=(kc == 0), stop=(kc == DC - 1))
                    nc.scalar.activation(bl[r][dc][:, g0:g0 + gn], ps[:, :gn],
                                         AF.Relu, bias=bias_sb[:, dc, 3 + ri:4 + ri])

        # ---------------- gate mix ----------------
        # h1_k = relu(sum_r m1[k,r]/r * B_r^expand + m1_b[k]);
        # gate = relu(sum_k m2[k] h1_k + m2_b).
        # k=0 on DVE (scalar_tensor_tensor chain); k=1,2 and the gate combine
        # on PE as scaled-identity accumulating matmuls, bias folded into the
        # ACT relu. Expansion = stride-0 rhs views.
        gate = []
        for dc in range(DC):
            ev = [b1_sb[dc][:]]
            for r in (2, 4, 10, 25):
                ev.append(bl[r][dc][:, :, None].to_broadcast([DCS, T // r, r]))

            def ev_chunk(ri, t0, tn):
                r = RANGES[ri]
                if r == 1:
                    return b1_sb[dc][:, t0:t0 + tn]
                return bl[r][dc][:, t0 // r:(t0 + tn) // r, None] \
                    .to_broadcast([DCS, tn // r, r])

            h1 = []
            # k = 0 on DVE
            acc = p_mix.tile([DCS, T], DT, tag="h1_0", name="h1_0")
            nc.vector.scalar_tensor_tensor(
                acc[:], ev[0], sc(SC_M1),
                scal_dt_sb[0:DCS, SC_M1B:SC_M1B + 1].to_broadcast([DCS, T]),
                op0=OP.mult, op1=OP.add)
            for ri in range(1, 5):
                nc.vector.scalar_tensor_tensor(
                    acc[:], ev[ri], sc(SC_M1 + ri), acc[:],
                    op0=OP.mult, op1=OP.add)
            nc.scalar.activation(acc[:], acc[:], AF.Relu)
            h1.append(acc)
            # k = 1, 2 on PE
            for k in (1, 2):
                acc = p_mix.tile([DCS, T], DT, tag=f"h1_{k}", name=f"h1_{k}")
                for t0 in range(0, T, N_MM):
                    ps = pp500.tile([DCS, N_MM], F32, tag="mm")
                    for ri in range(5):
                        nc.tensor.matmul(ps[:], mI[:, 5 * k + ri, :],
                                         ev_chunk(ri, t0, N_MM),
                                         start=(ri == 0), stop=(ri == 4))
                    nc.scalar.activation(acc[:, t0:t0 + N_MM], ps[:], AF.Relu,
                                         bias=sc(SC_M1B + k))
                h1.append(acc)
            # gate combine on PE
            g_acc = p_mix.tile([DCS, T], DT, tag="gate")
            for t0 in range(0, T, N_MM):
                ps = pp500.tile([DCS, N_MM], F32, tag="mm")
                for k in range(3):
                    nc.tensor.matmul(ps[:], mI[:, 15 + k, :],
                                     h1[k][:, t0:t0 + N_MM],
                                     start=(k == 0), stop=(k == 2))
                nc.scalar.activation(g_acc[:, t0:t0 + N_MM], ps[:], AF.Relu,
                                     bias=sc(SC_M2B))
            gate.append(g_acc)

        # ---------------- MRU scan + encode ----------------
        encT = []
        for dc in range(DC):
            gz = p_mix.tile([DCS, T], DT, tag="gz", name="gz")
            nc.gpsimd.tensor_tensor(gz[:], gate[dc][:], z_sb[dc][:], op=OP.mult)
            nc.vector.tensor_sub(z_sb[dc][:], z_sb[dc][:], gz[:])  # (1-g)z
            c_t = p_mix.tile([DCS, T], DT, tag="c", name="c_t")
            nc.vector.tensor_tensor_scan(
                c_t[:], gate[dc][:], z_sb[dc][:], 0.0, op0=OP.mult, op1=OP.add)
            nc.vector.tensor_mul(o_sb[dc][:], o_sb[dc][:], c_t[:])
            encT.append(o_sb[dc])

        # ---------------- keys1T ----------------
        k1T = small.tile([DCS, DC, TQ], DT, tag="k1T")
        for dc in range(DC):
            ps = ppatt.tile([DCS, TQ], F32, tag="work")
            for kc in range(DC):
                nc.tensor.matmul(ps[:], w_f1_sb[:, kc, dc * DCS:(dc + 1) * DCS],
                                 qT[:, kc, :], start=(kc == 0), stop=(kc == DC - 1))
            nc.scalar.copy(k1T[:, dc, :], ps[:])

        # ---------------- A2/A3 and QK ----------------
        aTs = []
        for fi, w_f_sb in enumerate((w_f2_sb, w_f3_sb)):
            a_ps = ppatt.tile([TQ, DIM], F32, tag="work")
            for kc in range(DC):
                nc.tensor.matmul(a_ps[:], qT[:, kc, :], w_f_sb[:, kc, :],
                                 start=(kc == 0), stop=(kc == DC - 1))
            a_sb = small.tile([TQ, DIM], DT, tag="a_sb")
            nc.vector.tensor_copy(a_sb[:], a_ps[:])
            aT = small.tile([DCS, DC, TQ], DT, tag=f"aT{fi}")
            for dc in range(DC):
                tp = pptr.tile([DCS, 128], DT, tag="work")
                nc.tensor.transpose(tp[:, :TQ], a_sb[:, dc * DCS:(dc + 1) * DCS],
                                    ident[:TQ, :TQ])
                nc.vector.tensor_copy(aT[:, dc, :], tp[:, :TQ])
            aTs.append(aT)

        qk_ps = ppacc.tile([TQ, 128], F32, tag="acc")
        for fi in range(2):
            for o in range(4):
                gcol = 16 * (4 * fi + o)
                for kc in range(DC):
                    nc.tensor.matmul(qk_ps[:, gcol:gcol + 16],
                                     aTs[fi][:, kc, :], oT[:, kc, o, :],
                                     start=(kc == 0), stop=(kc == DC - 1))
        qk_sb = small.tile([TQ, 128], DT, tag="qk_sb")
        nc.vector.tensor_copy(qk_sb[:], qk_ps[:])

        # ---------------- attention stream over t chunks ----------------
        pb_ps = ppacc.tile([128, 8], F32, tag="acc")
        for c in range(NTCH):
            pc = TCH[c]
            s1 = ppatt.tile([128, TQ], F32, tag="work")
            for dc in range(DC):
                nc.tensor.matmul(s1[:pc, :], encT[dc][:, c * 128:c * 128 + pc],
                                 k1T[:, dc, :], start=(dc == 0),
                                 stop=(dc == DC - 1))
            e1 = small.tile([128, TQ], DT, tag="e1")
            z1 = small.tile([128, 2], F32, tag="z1")
            nc.scalar.activation(e1[:pc, :], s1[:pc, :], AF.Exp,
                                 accum_out=z1[:pc, 0:1])
            nc.vector.reciprocal(z1[:pc, 1:2], z1[:pc, 0:1])
            tp = pptr.tile([TQ, 128], DT, tag="work")
            nc.tensor.transpose(tp[:, :pc], e1[:pc, :], ident[:pc, :pc])
            e1T = small.tile([TQ, 128], DT, tag="e1Ts")
            nc.vector.tensor_copy(e1T[:, :pc], tp[:, :pc])
            u2 = ppatt.tile([128, 128], F32, tag="work")
            nc.tensor.matmul(u2[:pc, :], e1T[:, :pc], qk_sb[:],
                             start=True, stop=True)
            e2 = small.tile([128, 128], F32, tag="e2")
            nc.scalar.activation(e2[:pc, :], u2[:pc, :], AF.Exp,
                                 scale=z1[:pc, 1:2])
            z2 = small.tile([128, 16], F32, tag="z2")
            nc.vector.tensor_reduce(z2[:pc, 0:8],
                                    e2[:pc, :].rearrange("p (g w) -> p g w", w=16),
                                    AX.X, OP.add)
            nc.vector.reciprocal(z2[:pc, 8:16], z2[:pc, 0:8])
            nc.tensor.matmul(pb_ps[:, :], e2[:pc, :], z2[:pc, 8:16],
                             start=(c == 0), stop=(c == NTCH - 1))

        # ---------------- answer vectors ----------------
        pb_sb = small.tile([128, 8], DT, tag="pb_sb")
        nc.vector.tensor_copy(pb_sb[:], pb_ps[:])
        ans_ps = ppacc.tile([DCS, 24], F32, tag="acc")
        for g in range(8):
            fi, o = g // 4, g % 4
            pb16 = small.tile([TO, 1], DT, tag="pb16")
            nc.sync.dma_start(pb16[:], pb_sb[16 * g:16 * g + 16, g:g + 1])
            for dc in range(DC):
                j = fi * 3 + dc
                nc.tensor.matmul(ans_ps[:, j * 4 + o:j * 4 + o + 1],
                                 og[o][:, dc * DCS:(dc + 1) * DCS], pb16[:],
                                 start=True, stop=True)
        # 1/T of the mean-over-t lands here (cheaper than scaling rz2 per chunk)
        nc.vector.tensor_scalar_mul(
            ans_sb[:, b, :, :].rearrange("p j o -> p (j o)"), ans_ps[:], 1.0 / T)

    # ---------------- final MLP (both batches together) ----------------
    h_ps = ppatt.tile([75, 8], F32, tag="work")
    for j in range(6):
        # rhs columns = (b, o) pairs for chunk j of the 600-dim ans vector
        rhs = ans_sb[:, :, j, :]
        nc.tensor.matmul(h_ps[:], w_as1_sb[:, j, :], rhs,
                         start=(j == 0), stop=(j == 5))
    h_sb = small.tile([75, 8], F32, tag="h_sb")
    nc.scalar.activation(h_sb[:], h_ps[:], AF.Relu, bias=b_as1_sb[:])
    s_ps = ppacc.tile([8, 1], F32, tag="acc")
    nc.tensor.matmul(s_ps[:], h_sb[:], w_as2_sb[:], start=True, stop=True)
    s_sb = small.tile([8, 1], F32, tag="s_sb")
    nc.scalar.activation(s_sb[:], s_ps[:], AF.Identity,
                         bias=scal_sb[0:8, SC_AS2B:SC_AS2B + 1])
    nc.sync.dma_start(out[:].rearrange("b o -> (b o)")[:, None], s_sb[:])


# ---------------------------------------------------------------------------
# host side
# ---------------------------------------------------------------------------

_CACHE = {}


def _get_nc():
    if "nc" not in _CACHE:
        _CACHE["nc"] = _build_program()
    return _CACHE["nc"]


def _prep_core_inputs(inputs, core):
    b0 = core * BPC
    sl = slice(b0, b0 + BPC)
    f = np.asarray
    prep = _CACHE.get("prep_shared")
    if prep is None:
        # core-independent tensors, computed once per kernel() call set
        Wz, Wo = f(inputs["Wz"]), f(inputs["Wo"])
        ceW = f(inputs["ce_W"])
        prep = {
            "emb": f(inputs["emb"]).astype(NPDT),
            "w_art": np.ascontiguousarray(
                np.concatenate([Wz.T, Wo.T, ceW[0].T], axis=1)).astype(NPDT),
            "w_ce": np.ascontiguousarray(
                ceW[1:].transpose(0, 2, 1)).astype(NPDT),
            "w_f1": np.ascontiguousarray(f(inputs["f1_W"]).T).astype(NPDT),
            # s2 = aoq @ f2W @ opt^T, so f2/f3 go in UNtransposed
            # (f1 builds keys1^T = f1W @ q^T and does need the transpose)
            "w_f2": np.ascontiguousarray(f(inputs["f2_W"])).astype(NPDT),
            "w_f3": np.ascontiguousarray(f(inputs["f3_W"])).astype(NPDT),
            "w_as1": np.ascontiguousarray(f(inputs["as1_W"]).T).astype(np.float32),
            "w_as2": np.ascontiguousarray(f(inputs["as2_W"]).T).astype(np.float32),
            "biases": np.stack(
                [f(inputs["bz"]), f(inputs["bo"]),
                 *[f(inputs["ce_b"])[i] for i in range(5)],
                 f(inputs["f1_b"]), f(inputs["f2_b"]), f(inputs["f3_b"])],
                axis=1).astype(np.float32),
            "b_as1": f(inputs["as1_b"])[:, None].astype(np.float32),
        }
        scal = np.zeros((128, SC_NCOL), np.float32)
        m1 = f(inputs["mr1_W"])
        for k in range(3):
            for ri, r in enumerate(RANGES):
                scal[:, SC_M1 + 5 * k + ri] = m1[k, ri] / r
        scal[:, SC_M1B:SC_M1B + 3] = f(inputs["mr1_b"])[None, :]
        scal[:, SC_M2:SC_M2 + 3] = f(inputs["mr2_W"])[0][None, :]
        scal[:, SC_M2B] = f(inputs["mr2_b"])[0]
        scal[:, SC_AS2B] = f(inputs["as2_b"])[0]
        prep["scal"] = scal
        prep["scal_dt"] = scal.astype(NPDT)
        _CACHE["prep_shared"] = prep

    d = dict(prep)
    d["art_idx"] = f(inputs["article_in"])[sl].astype(np.int32)
    d["q_idx"] = f(inputs["question_in"])[sl].astype(np.int32)
    d["opt_idx"] = np.stack(
        [f(inputs[f"option{i}_in"])[sl] for i in (1, 2, 3, 4)],
        axis=1).astype(np.int32)
    return d


def _get_runner():
    """jit-compiled 8-core runner, built once per process."""
    if "runner" in _CACHE:
        return _CACHE["runner"]
    import jax
    from jax.sharding import Mesh, PartitionSpec
    from jax.experimental.shard_map import shard_map
    from concourse.bass2jax import (_bass_exec_p, install_neuronx_cc_hook,
                                    partition_id_tensor)

    install_neuronx_cc_hook()
    nc = _get_nc()
    pid_name = nc.partition_id_tensor.name if nc.partition_id_tensor else None

    in_names, out_names, out_avals, zero_outs = [], [], [], []
    for alloc in nc.m.functions[0].allocations:
        if not isinstance(alloc, mybir.MemoryLocationSet):
            continue
        name = alloc.memorylocations[0].name
        if alloc.kind == "ExternalInput":
            if name != pid_name:
                in_names.append(name)
        elif alloc.kind == "ExternalOutput":
            out_names.append(name)
            shape = tuple(alloc.tensor_shape)
            dtype = mybir.dt.np(alloc.dtype)
            out_avals.append(jax.core.ShapedArray(shape, dtype))
            zero_outs.append(np.zeros(shape, dtype))
    n_params = len(in_names)
    all_in_names = in_names + out_names
    if pid_name is not None:
        all_in_names = all_in_names + [pid_name]

    def _body(*args):
        operands = list(args)
        if pid_name is not None:
            operands.append(partition_id_tensor())
        outs = _bass_exec_p.bind(
            *operands, out_avals=tuple(out_avals), in_names=tuple(all_in_names),
            out_names=tuple(out_names), lowering_input_output_aliases=(),
            sim_require_finite=True, sim_require_nnan=True, nc=nc)
        return tuple(outs)

    devices = jax.devices()[:NCORES]
    mesh = Mesh(np.asarray(devices), ("core",))
    in_specs = (PartitionSpec("core"),) * (n_params + len(out_names))
    out_specs = (PartitionSpec("core"),) * len(out_names)
    sharded = jax.jit(shard_map(_body, mesh=mesh, in_specs=in_specs,
                                out_specs=out_specs, check_rep=False),
                      keep_unused=True)

    _CACHE["runner"] = (sharded, in_names, out_names, zero_outs)
    return _CACHE["runner"]


def run_cores(per_core_inputs):
    """per_core_inputs: list of 8 dicts name->np array. Returns out dicts."""
    sharded, in_names, out_names, zero_outs = _get_runner()
    concat_in = [np.concatenate([per_core_inputs[c][n] for c in range(NCORES)],
                                axis=0) for n in in_names]
    concat_zeros = [np.concatenate([z] * NCORES, axis=0) for z in zero_outs]
    outs = sharded(*concat_in, *concat_zeros)
    result = []
    for c in range(NCORES):
        d = {}
        for i, n in enumerate(out_names):
            arr = np.asarray(outs[i])
            per = arr.shape[0] // NCORES
            d[n] = arr[c * per:(c + 1) * per]
        result.append(d)
    return result


def prepare_device_args(per_core_inputs):
    """device_put the concatenated inputs once, for repeated timed runs."""
    import jax
    from jax.sharding import Mesh, PartitionSpec, NamedSharding
    sharded, in_names, out_names, zero_outs = _get_runner()
    devices = jax.devices()[:NCORES]
    mesh = Mesh(np.asarray(devices), ("core",))
    sh = NamedSharding(mesh, PartitionSpec("core"))
    concat_in = [np.concatenate([per_core_inputs[c][n] for c in range(NCORES)],
                                axis=0) for n in in_names]
    concat_zeros = [np.concatenate([z] * NCORES, axis=0) for z in zero_outs]
    args = [jax.device_put(a, sh) for a in concat_in + concat_zeros]
    jax.block_until_ready(args)
    return args


def run_prepared(dev_args):
    sharded, in_names, out_names, zero_outs = _get_runner()
    outs = sharded(*dev_args)
    import jax
    jax.block_until_ready(outs)
    return outs


def kernel(**inputs):
    _CACHE.pop("prep_shared", None)
    per_core = [_prep_core_inputs(inputs, c) for c in range(NCORES)]
    res = run_cores(per_core)
    out = np.concatenate([res[c]["scores"] for c in range(NCORES)], axis=0)
    return out.astype(np.float32)
